# revision 25
# baseline (speedup 1.0000x reference)
"""2-layer GCN (GCNConv -> ReLU -> BN -> GCNConv -> ReLU) on 8 trn2 NeuronCores.

Strategy (single SPMD program on all 8 cores):
  - Nodes ranked by degree (ascending), dealt round-robin to cores; node n
    lives at shard row pos(n) of core owner(n), global table row
    row(n) = owner*SH + pos.
  - P1 is SHARDED: each core computes h1*dinv only for its own 12.8k rows
    (x @ W1 reads 26MB instead of 205MB), writes a bf16 shard, then one
    AllGather (Shared-address DRAM, NRT shared-output fast path, ~30us)
    builds the full layer-1 table. Same for the layer-2 table after L1.
  - Neighbor aggregation uses gpsimd.dma_gather (SWDGE): one instruction
    gathers up to 1024 rows (the ucode cap) at 256B/descriptor. Tables are
    packed 4 nodes per 256B group row so the int16 gather index covers all
    102400 rows (25600 groups < 32768). Per dst-tile the gather fetches the
    whole 4-node group per slot; a precomputed {0,1} bf16 mask (nonzero at
    the slot's sub-row) is multiplied in-place and a strided reduce_sum
    collapses slots -> [128,32] f32. Pad slots have mask 0, so per-tile slot
    counts only pad to the tile max indegree (~5%), and both layers share
    identical idx/mask arrays (self-loop terms are added separately from
    per-core f32 copies h1f/h2f via plain DMA).
  - 4 SWDGE queues round-robin the gathers; descriptor-ring scratch 48KB.
  - Epilogues are BATCHED 4 dst-tiles at a time (T_all = 100 = 25*4):
    elementwise ops run on [128, 128] buffers, and the layer-1
    relu->transpose->@W2' chain uses one PE transpose plus one matmul
    against a block-diagonal [128,128] W2' so one PE pass serves 4 tiles.
    This cuts per-tile cross-engine hops ~3x.
  - After each AllGather a tiny fence AllGather runs on the in-order
    collective lane; completion implies every core's big AllGather finished
    (guards the NRT shared-output write-split race). Its zero payload is
    added in-place (int16) onto the gather index tile, so every subsequent
    dma_gather RAW-depends on the fence.
  - BN (eval mode) folded into W2' = diag(s) @ W2 and c2 = t @ W2 on host.

Host does only index/graph-structure preprocessing; all tensor math runs on
device.
"""

import numpy as np
import ml_dtypes

import concourse.bass as bass
import concourse.bacc as bacc
import concourse.mybir as mybir
import concourse.tile as tile
from concourse.bass_utils import run_bass_kernel_spmd

F32 = mybir.dt.float32
BF16 = mybir.dt.bfloat16
I16 = mybir.dt.int16
BF16NP = ml_dtypes.bfloat16

C = 8          # cores
P = 128        # partitions
H = 32         # hidden dim
D = 512        # input dim
BN_EPS = 1e-5
KU = 8         # k-units (slots) per gather instruction: 8*128 = 1024 idx cap
TB = 4         # dst-tiles per batched epilogue


def _plan(n_nodes, edge_index):
    """Host-side graph preprocessing -> per-core index/mask arrays."""
    src = np.asarray(edge_index[0], dtype=np.int64)
    dst = np.asarray(edge_index[1], dtype=np.int64)

    deg = np.bincount(dst, minlength=n_nodes).astype(np.float32) + 1.0
    dinv = (1.0 / np.sqrt(deg)).astype(np.float32)

    per = n_nodes // C
    SH = -(-per // 512) * 512
    T_real = (per + P - 1) // P
    T_all = SH // P

    order = np.argsort(deg, kind="stable")
    owner = np.empty(n_nodes, dtype=np.int64)
    pos = np.empty(n_nodes, dtype=np.int64)
    ranks = np.arange(n_nodes)
    owner[order] = ranks % C
    pos[order] = ranks // C
    assert pos.max() == per - 1

    row = owner * SH + pos
    grp = (row >> 2).astype(np.int16)
    sub = (row & 3).astype(np.int64)

    e_owner = owner[dst]
    e_pos = pos[dst]
    counts = np.zeros((C, per), dtype=np.int64)
    np.add.at(counts, (e_owner, e_pos), 1)

    K_list = []
    for t in range(T_real):
        lo, hi = t * P, min((t + 1) * P, per)
        K_list.append(max(1, int(counts[:, lo:hi].max())))

    # gather-batches of GB adjacent tiles (degree-sorted -> similar K);
    # every tile in batch bb padded to Kb = batch max (pad idx 0, mask 0)
    GB = 2
    assert T_real % GB == 0
    NGB = T_real // GB
    Kb = np.array([max(K_list[bb * GB:(bb + 1) * GB]) for bb in range(NGB)],
                  dtype=np.int64)
    boffs = np.concatenate([[0], np.cumsum(GB * Kb)]).astype(np.int64)
    totK = int(boffs[-1])          # padded slot total
    # padded slot offset of each tile
    soff = np.array([boffs[t // GB] + (t % GB) * Kb[t // GB]
                     for t in range(T_real)], dtype=np.int64)

    # per-edge slot assignment within (core, dst)
    eorder = np.lexsort((src, e_pos, e_owner))
    so, sp, ss = e_owner[eorder], e_pos[eorder], src[eorder]
    gkey = so * per + sp
    newg = np.ones(len(gkey), dtype=bool)
    newg[1:] = gkey[1:] != gkey[:-1]
    gstart = np.where(newg)[0]
    slot = np.arange(len(gkey)) - np.repeat(
        gstart, np.diff(np.concatenate([gstart, [len(gkey)]])))
    tt = sp // P
    lane = sp % P

    # idx: per-instruction wrapped int16 blocks (instructions span the whole
    # batch's GB*Kb k-units); mask: duplicated pairs for DVE 2x
    idx16 = np.zeros((C, 16, 8 * totK), np.int16)
    mask = np.zeros((C, P, 4 * totK), np.float32)
    ebb = tt // GB
    kk = (tt % GB) * Kb[ebb] + slot           # within-batch k-unit
    j8 = kk % KU
    chunk = kk // KU
    i_local = j8 * P + lane
    col = 8 * boffs[ebb] + chunk * (8 * KU) + i_local // 16
    rr = i_local % 16
    idx16[so, rr, col] = grp[ss]
    mask[so, lane, 4 * (soff[tt] + slot) + sub[ss]] = 1.0
    idx_arr = np.tile(idx16, (1, 8, 1))
    mask_arr = np.repeat(mask, 2, axis=2).astype(BF16NP)

    # per-instruction chunk table: (batch, col0, ku, k0)
    chunks = []
    for bb in range(NGB):
        nk = GB * int(Kb[bb])
        nfull, rem = divmod(nk, KU)
        for h in range(nfull):
            chunks.append((bb, 8 * boffs[bb] + h * 8 * KU, KU, h * KU))
        if rem:
            chunks.append((bb, 8 * boffs[bb] + nfull * 8 * KU, rem,
                           nfull * KU))

    # dinv per core in shard order -> [P, T_all]
    nodes_by_cp = np.full((C, per), -1, dtype=np.int64)
    nodes_by_cp[owner, pos] = np.arange(n_nodes)
    dinv_s = np.zeros((C, P, T_all), dtype=np.float32)
    for c in range(C):
        fulls = np.zeros(SH, np.float32)
        fulls[:per] = dinv[nodes_by_cp[c]]
        dinv_s[c] = fulls.reshape(T_all, P).T

    meta = dict(per=per, SH=SH, T_real=T_real, T_all=T_all, K_list=K_list,
                GB=GB, NGB=NGB, Kb=Kb, boffs=boffs, soff=soff,
                totK=totK, maxK=max(K_list), chunks=chunks,
                nodes_by_cp=nodes_by_cp)
    return idx_arr, mask_arr, dinv_s, meta


def _build_nc(n_nodes, meta, phases=("p1", "ag1", "l1", "ag2", "l2"), reps=1):
    phases = set(phases)
    SH, T_real, T_all = meta["SH"], meta["T_real"], meta["T_all"]
    totK, maxK = meta["totK"], meta["maxK"]
    TAB = C * SH

    nc = bacc.Bacc("TRN2", target_bir_lowering=False, debug=False,
                   num_devices=C, dynamic_dma_scratch_size=49152,
                   num_swdge_queues=4)
    xT = nc.dram_tensor("xT", [D, SH], F32, kind="ExternalInput").ap()
    w1 = nc.dram_tensor("w1", [D, H], F32, kind="ExternalInput").ap()
    w2b = nc.dram_tensor("w2b", [P, P], F32, kind="ExternalInput").ap()
    b1r = nc.dram_tensor("b1r", [P, TB * H], F32, kind="ExternalInput").ap()
    b2r = nc.dram_tensor("b2r", [P, TB * H], F32, kind="ExternalInput").ap()
    c2r = nc.dram_tensor("c2r", [P, TB * H], F32, kind="ExternalInput").ap()
    ident = nc.dram_tensor("ident", [P, P], F32, kind="ExternalInput").ap()
    dinvs = nc.dram_tensor("dinvs", [P, T_all], F32, kind="ExternalInput").ap()
    idx = nc.dram_tensor("idx", [P, 8 * totK], I16, kind="ExternalInput").ap()
    msk = nc.dram_tensor("msk", [P, 8 * totK], BF16, kind="ExternalInput").ap()
    out = nc.dram_tensor("out", [SH, H], F32, kind="ExternalOutput").ap()

    with tile.TileContext(nc) as tc:
        with (
            tc.tile_pool(name="cst", bufs=1) as cst,
            tc.tile_pool(name="sb", bufs=3) as sb,
            tc.tile_pool(name="gp", bufs=2) as gp,
            tc.tile_pool(name="fp", bufs=2) as fp,
            tc.tile_pool(name="xtp", bufs=3) as xtp,
            tc.tile_pool(name="ps", bufs=2, space="PSUM") as ps,
            tc.tile_pool(name="dram", bufs=1, space="DRAM") as dram,
        ):
            # constants
            w1t = cst.tile([P, 4 * H], F32)
            for f in range(4):
                nc.sync.dma_start(w1t[:, f * H:(f + 1) * H],
                                  w1[f * P:(f + 1) * P, :])
            w2bt = cst.tile([P, P], F32)
            nc.sync.dma_start(w2bt[:], w2b[:, :])
            b1t = cst.tile([P, TB * H], F32)
            nc.sync.dma_start(b1t[:], b1r[:, :])
            b2t = cst.tile([P, TB * H], F32)
            nc.sync.dma_start(b2t[:], b2r[:, :])
            c2t = cst.tile([P, TB * H], F32)
            nc.sync.dma_start(c2t[:], c2r[:, :])
            idt = cst.tile([P, P], F32)
            nc.sync.dma_start(idt[:], ident[:, :])
            dst_ = cst.tile([P, T_all], F32)
            nc.sync.dma_start(dst_[:], dinvs[:, :])
            ixt = cst.tile([P, 8 * totK], I16)
            nc.sync.dma_start(ixt[:], idx[:, :])
            mkt = cst.tile([P, 8 * totK], BF16)
            nc.sync.dma_start(mkt[:], msk[:, :])
            fz = cst.tile([1, H], BF16)
            nc.vector.memset(fz[:], 0.0)

            h1s = dram.tile([SH, H], BF16)
            h2s = dram.tile([SH, H], BF16)
            fsrc = dram.tile([1, H], BF16)
            nc.sync.dma_start(fsrc[:], fz[:])

            env = dict(locals())
            env["dram"] = dram
            for r in range(reps):
                tab1 = dram.tile([TAB, H], BF16, addr_space="Shared",
                                 tag=f"tab1_{r}")
                tab2 = dram.tile([TAB, H], BF16, addr_space="Shared",
                                 tag=f"tab2_{r}")
                env["tab1"], env["tab2"] = tab1, tab2
                env["rep"] = r
                _body(nc, tc, phases, meta, env)

    nc.compile()
    return nc


def _body(nc, tc, phases, meta, env):
    SH, T_real, T_all = meta["SH"], meta["T_real"], meta["T_all"]
    K_list, maxK = meta["K_list"], meta["maxK"]
    chunks = meta["chunks"]
    TAB = C * SH
    NST = SH // 512
    NB = T_all // TB           # batched epilogue groups

    xT = env["xT"]; out = env["out"]
    sb = env["sb"]; gp = env["gp"]; ps = env["ps"]
    fp = env["fp"]; xtp = env["xtp"]
    GB, NGB, Kb, boffs = meta["GB"], meta["NGB"], meta["Kb"], meta["boffs"]
    tab1 = env["tab1"]; tab2 = env["tab2"]
    h1s = env["h1s"]; h2s = env["h2s"]
    fsrc = env["fsrc"]
    w1t = env["w1t"]; w2bt = env["w2bt"]; b1t = env["b1t"]; b2t = env["b2t"]
    c2t = env["c2t"]; idt = env["idt"]; dst_ = env["dst_"]
    ixt = env["ixt"]; mkt = env["mkt"]

    qn = [0]

    def gather_batch(g, tabg, bb):
        g3 = g[:].rearrange("p (k e) -> p k e", e=P)
        for (ct, col0, ku, k0) in chunks:
            if ct != bb:
                continue
            ni = P * ku
            nc.gpsimd.dma_gather(
                g3[:, k0:k0 + ku, :], tabg,
                ixt[:, col0:col0 + 8 * ku], ni, ni, P,
                queue_num=qn[0] % 4)
            qn[0] += 1

    def masked_fold(g, redb, bb, j_base):
        """mask-mult (DVE 2x) + 2 packed folds for a whole gather batch,
        then one short strided reduce per tile."""
        nk = GB * int(Kb[bb])
        g4 = g[:, :nk * P].rearrange("p (j g e) -> p j g e", g=16, e=2)
        nc.vector.tensor_mul(
            g4, g4,
            mkt[:, 8 * boffs[bb]:8 * (boffs[bb] + nk)]
               .rearrange("p (j e) -> p j e", e=2).unsqueeze(2)
               .broadcast_to([P, 4 * nk, 16, 2]))
        gb_ = g[:, :nk * P].rearrange("p (j b f) -> p j b f", b=4, f=H)
        t1 = fp.tile([P, GB * maxK * 2 * H], BF16, tag="t1")
        t1v = t1[:, :nk * 2 * H].rearrange("p (j b f) -> p j b f", b=2, f=H)
        nc.vector.tensor_add(t1v, gb_[:, :, 0:2, :], gb_[:, :, 2:4, :])
        s1 = fp.tile([P, GB * maxK * H], BF16, tag="s1")
        s1v = s1[:, :nk * H].rearrange("p (j f) -> p j f", f=H)
        nc.vector.tensor_add(s1v, t1v[:, :, 0, :], t1v[:, :, 1, :])
        kb = int(Kb[bb])
        for j in range(GB):
            nc.vector.reduce_sum(
                out=redb[:, (j_base + j) * H:(j_base + j + 1) * H],
                in_=s1[:, j * kb * H:(j + 1) * kb * H]
                    .rearrange("p (j f) -> p f j", f=H),
                axis=mybir.AxisListType.X)

    def bcast_dinv(b):
        return (dst_[:, b * TB:(b + 1) * TB].unsqueeze(-1)
                .broadcast_to([P, TB, H]))

    def batch_rows(dr, b):
        """[p, j, f] DRAM-side AP of rows for tile-batch b."""
        return dr[b * TB * P:(b + 1) * TB * P, :].rearrange(
            "(j p) f -> p j f", j=TB)

    def v3(sbuf_ap):
        return sbuf_ap.rearrange("p (j f) -> p j f", f=H)

    def fence(tag):
        """Tiny AllGather; completion implies every core's preceding
        collective finished (in-order collective lane). The in-place int16
        add of its zero payload onto ixt makes later gathers RAW-depend on
        it."""
        fb = env["dram"].tile([C, H], BF16, tag=f"fb_{tag}")
        nc.gpsimd.collective_compute(
            "AllGather", mybir.AluOpType.bypass,
            replica_groups=[list(range(C))],
            ins=[fsrc.opt()], outs=[fb.opt()])
        fbt = sb.tile([P, 2], BF16, tag=f"fbt_{tag}")
        nc.sync.dma_start(fbt[:], fb[0:1, 0:2].to_broadcast((P, 2)))
        nc.vector.tensor_add(
            ixt[:], ixt[:],
            fbt[:, 0:1].bitcast(I16).broadcast_to([P, 8 * meta["totK"]]))

    # ---- P1 (sharded): h1 = (x @ W1) * dinv for my 12.8k rows ----
    for st in range(NST if "p1" in phases else 0):
        xt = xtp.tile([P, 4 * D], F32, tag="xt")
        for f in range(4):
            nc.sync.dma_start(
                xt[:, f * D:(f + 1) * D],
                xT[f * P:(f + 1) * P, st * 512:(st + 1) * 512])
        ht = sb.tile([P, 4 * H], BF16, tag="ht")
        for g4 in range(4):
            pp = ps.tile([P, H], F32, tag="p1ps")
            for f in range(4):
                nc.tensor.matmul(
                    pp[:],
                    lhsT=xt[:, f * D + g4 * P: f * D + (g4 + 1) * P],
                    rhs=w1t[:, f * H:(f + 1) * H],
                    start=(f == 0), stop=(f == 3))
            g = st * 4 + g4
            nc.scalar.activation(ht[:, g4 * H:(g4 + 1) * H], pp[:],
                                 mybir.ActivationFunctionType.Copy,
                                 scale=dst_[:, g:g + 1])
        nc.sync.dma_start(batch_rows(h1s, st), v3(ht[:]))

    # ---- AG1 + fence ----
    if "ag1" in phases:
        nc.gpsimd.collective_compute(
            "AllGather", mybir.AluOpType.bypass,
            replica_groups=[list(range(C))],
            ins=[h1s.opt()], outs=[tab1.opt()])
        fence(f"1_{env['rep']}")

    # ---- L1: aggregate + batched epilogue (BN folded into W2', c2) ----
    tab1g = tab1[:].rearrange("(g r) f -> g (r f)", r=4)
    NGPB = TB // GB
    for b in range(NB if "l1" in phases else 0):
        redb = sb.tile([P, TB * H], F32, tag="redb")
        nzero = 0
        for gi in range(NGPB):
            bb = b * NGPB + gi
            if bb >= NGB:
                nzero += GB
                continue
            g = gp.tile([P, GB * maxK * P], BF16, tag="g")
            gather_batch(g, tab1g, bb)
            masked_fold(g, redb, bb, gi * GB)
        if nzero:
            nc.vector.memset(redb[:, (TB - nzero) * H:], 0.0)
        sf = sb.tile([P, TB * H], BF16, tag="sf")
        nc.sync.dma_start(v3(sf[:]), batch_rows(h1s, b))
        nc.vector.tensor_add(redb[:], redb[:], sf[:])
        nc.vector.tensor_mul(
            redb[:].rearrange("p (j f) -> p j f", f=H),
            redb[:].rearrange("p (j f) -> p j f", f=H), bcast_dinv(b))
        nc.vector.tensor_add(redb[:], redb[:], b1t[:])
        nc.vector.tensor_scalar_max(redb[:], redb[:], 0.0)
        pt = ps.tile([P, P], F32, tag="pst")
        nc.tensor.transpose(pt[:], redb[:], idt[:])
        rt = sb.tile([P, P], F32, tag="rt")
        nc.scalar.activation(rt[:], pt[:],
                             mybir.ActivationFunctionType.Copy)
        p2 = ps.tile([P, P], F32, tag="ps2")
        nc.tensor.matmul(p2[:], lhsT=rt[:], rhs=w2bt[:],
                         start=True, stop=True)
        h2b = sb.tile([P, TB * H], F32, tag="h2b")
        nc.vector.tensor_add(h2b[:], p2[:], c2t[:])
        nc.vector.tensor_mul(
            h2b[:].rearrange("p (j f) -> p j f", f=H),
            h2b[:].rearrange("p (j f) -> p j f", f=H), bcast_dinv(b))
        h2c = sb.tile([P, TB * H], BF16, tag="h2c")
        nc.vector.tensor_copy(h2c[:], h2b[:])
        nc.sync.dma_start(batch_rows(h2s, b), v3(h2c[:]))

    # ---- AG2 + fence ----
    if "ag2" in phases:
        nc.gpsimd.collective_compute(
            "AllGather", mybir.AluOpType.bypass,
            replica_groups=[list(range(C))],
            ins=[h2s.opt()], outs=[tab2.opt()])
        fence(f"2_{env['rep']}")

    # ---- L2: aggregate + batched epilogue ----
    tab2g = tab2[:].rearrange("(g r) f -> g (r f)", r=4)
    for b in range(NB if "l2" in phases else 0):
        redb = sb.tile([P, TB * H], F32, tag="redb2")
        nzero = 0
        for gi in range(NGPB):
            bb = b * NGPB + gi
            if bb >= NGB:
                nzero += GB
                continue
            g = gp.tile([P, GB * maxK * P], BF16, tag="g")
            gather_batch(g, tab2g, bb)
            masked_fold(g, redb, bb, gi * GB)
        if nzero:
            nc.vector.memset(redb[:, (TB - nzero) * H:], 0.0)
        sf = sb.tile([P, TB * H], BF16, tag="sf2")
        nc.sync.dma_start(v3(sf[:]), batch_rows(h2s, b))
        nc.vector.tensor_add(redb[:], redb[:], sf[:])
        nc.vector.tensor_mul(
            redb[:].rearrange("p (j f) -> p j f", f=H),
            redb[:].rearrange("p (j f) -> p j f", f=H), bcast_dinv(b))
        nc.vector.tensor_add(redb[:], redb[:], b2t[:])
        nc.vector.tensor_scalar_max(redb[:], redb[:], 0.0)
        ot = sb.tile([P, TB * H], F32, tag="ot")
        nc.vector.tensor_copy(ot[:], redb[:])
        nc.sync.dma_start(batch_rows(out, b), v3(ot[:]))


def _impl(x, edge_index, W1, b1, W2, b2, gamma, beta, run_mean, run_var,
          n_nodes):
    x = np.asarray(x, np.float32)
    W1 = np.asarray(W1, np.float32)
    b1 = np.asarray(b1, np.float32)
    W2 = np.asarray(W2, np.float32)
    b2 = np.asarray(b2, np.float32)
    gamma = np.asarray(gamma, np.float32)
    beta = np.asarray(beta, np.float32)
    run_mean = np.asarray(run_mean, np.float32)
    run_var = np.asarray(run_var, np.float32)

    idx_arr, mask_arr, dinv_s, meta = _plan(n_nodes, np.asarray(edge_index))
    per, SH = meta["per"], meta["SH"]

    s = gamma / np.sqrt(run_var + BN_EPS)
    t = beta - run_mean * s
    W2p = (W2 * s[:, None]).astype(np.float32)
    c2 = (t @ W2).astype(np.float32)

    w2blk = np.zeros((P, P), np.float32)
    for j in range(TB):
        w2blk[j * H:(j + 1) * H, j * H:(j + 1) * H] = W2p
    b1rep = np.tile(b1[None, :], (P, TB)).astype(np.float32)
    b2rep = np.tile(b2[None, :], (P, TB)).astype(np.float32)
    c2rep = np.tile(c2[None, :], (P, TB)).astype(np.float32)
    identv = np.eye(P, dtype=np.float32)

    nodes_by_cp = meta["nodes_by_cp"]
    in_maps = []
    for c in range(C):
        xs = np.zeros((SH, D), np.float32)
        xs[:per] = x[nodes_by_cp[c]]
        in_maps.append({
            "xT": np.ascontiguousarray(xs.T), "w1": W1, "w2b": w2blk,
            "b1r": b1rep, "b2r": b2rep, "c2r": c2rep, "ident": identv,
            "dinvs": np.ascontiguousarray(dinv_s[c]),
            "idx": np.ascontiguousarray(idx_arr[c]),
            "msk": np.ascontiguousarray(mask_arr[c]),
        })

    nc = _build_nc(n_nodes, meta)
    global _LAST_NC, _LAST_IN_MAPS, _LAST_META
    _LAST_NC, _LAST_IN_MAPS, _LAST_META = nc, in_maps, meta
    res = run_bass_kernel_spmd(nc, in_maps, core_ids=list(range(C))).results

    outf = np.zeros((n_nodes, H), np.float32)
    for c in range(C):
        outf[nodes_by_cp[c]] = res[c]["out"][:per]
    return outf


def kernel(x, edge_index, W1, b1, W2, b2, gamma, beta, run_mean, run_var):
    return _impl(x, edge_index, W1, b1, W2, b2, gamma, beta, run_mean,
                 run_var, n_nodes=100000)


# revision 26
# speedup vs baseline: 1.1642x; 1.1642x over previous
"""2-layer GCN (GCNConv -> ReLU -> BN -> GCNConv -> ReLU) on 8 trn2 NeuronCores.

Strategy (single SPMD program on all 8 cores):
  - Nodes ranked by degree (ascending), dealt round-robin to cores; node n
    lives at shard row pos(n) of core owner(n), global table row
    row(n) = owner*SH + pos.
  - P1 is SHARDED: each core computes h1*dinv only for its own 12.8k rows
    (x @ W1 reads 26MB instead of 205MB), writes a bf16 shard, then one
    AllGather (Shared-address DRAM, NRT shared-output fast path, ~30us)
    builds the full layer-1 table. Same for the layer-2 table after L1.
  - Neighbor aggregation uses gpsimd.dma_gather (SWDGE): one instruction
    gathers up to 1024 rows (the ucode cap) at 256B/descriptor. Tables are
    packed 4 nodes per 256B group row so the int16 gather index covers all
    102400 rows (25600 groups < 32768). Per dst-tile the gather fetches the
    whole 4-node group per slot; a precomputed {0,1} bf16 mask (nonzero at
    the slot's sub-row) is multiplied in-place and a strided reduce_sum
    collapses slots -> [128,32] f32. Pad slots have mask 0, so per-tile slot
    counts only pad to the tile max indegree (~5%), and both layers share
    identical idx/mask arrays (self-loop terms are added separately from
    per-core f32 copies h1f/h2f via plain DMA).
  - 4 SWDGE queues round-robin the gathers; descriptor-ring scratch 48KB.
  - Epilogues are BATCHED 4 dst-tiles at a time (T_all = 100 = 25*4):
    elementwise ops run on [128, 128] buffers, and the layer-1
    relu->transpose->@W2' chain uses one PE transpose plus one matmul
    against a block-diagonal [128,128] W2' so one PE pass serves 4 tiles.
    This cuts per-tile cross-engine hops ~3x.
  - After each AllGather a tiny fence AllGather runs on the in-order
    collective lane; completion implies every core's big AllGather finished
    (guards the NRT shared-output write-split race). Its zero payload is
    added in-place (int16) onto the gather index tile, so every subsequent
    dma_gather RAW-depends on the fence.
  - BN (eval mode) folded into W2' = diag(s) @ W2 and c2 = t @ W2 on host.

Host does only index/graph-structure preprocessing; all tensor math runs on
device.
"""

import numpy as np
import ml_dtypes

import concourse.bass as bass
import concourse.bacc as bacc
import concourse.mybir as mybir
import concourse.tile as tile
from concourse.bass_utils import run_bass_kernel_spmd

F32 = mybir.dt.float32
BF16 = mybir.dt.bfloat16
I16 = mybir.dt.int16
BF16NP = ml_dtypes.bfloat16

C = 8          # cores
P = 128        # partitions
H = 32         # hidden dim
D = 512        # input dim
BN_EPS = 1e-5
KU = 8         # k-units (slots) per gather instruction: 8*128 = 1024 idx cap
TB = 4         # dst-tiles per batched epilogue


def _plan(n_nodes, edge_index):
    """Host-side graph preprocessing -> per-core index/mask arrays."""
    src = np.asarray(edge_index[0], dtype=np.int64)
    dst = np.asarray(edge_index[1], dtype=np.int64)

    deg = np.bincount(dst, minlength=n_nodes).astype(np.float32) + 1.0
    dinv = (1.0 / np.sqrt(deg)).astype(np.float32)

    per = n_nodes // C
    SH = -(-per // 512) * 512
    T_real = (per + P - 1) // P
    T_all = SH // P

    order = np.argsort(deg, kind="stable")
    owner = np.empty(n_nodes, dtype=np.int64)
    pos = np.empty(n_nodes, dtype=np.int64)
    ranks = np.arange(n_nodes)
    owner[order] = ranks % C
    pos[order] = ranks // C
    assert pos.max() == per - 1

    row = owner * SH + pos
    grp = (row >> 2).astype(np.int16)
    sub = (row & 3).astype(np.int64)

    e_owner = owner[dst]
    e_pos = pos[dst]
    counts = np.zeros((C, per), dtype=np.int64)
    np.add.at(counts, (e_owner, e_pos), 1)

    K_list = []
    for t in range(T_real):
        lo, hi = t * P, min((t + 1) * P, per)
        K_list.append(max(1, int(counts[:, lo:hi].max())))

    # gather-batches of GB adjacent tiles (degree-sorted -> similar K);
    # every tile in batch bb padded to Kb = batch max (pad idx 0, mask 0)
    GB = 2
    assert T_real % GB == 0
    NGB = T_real // GB
    Kb = np.array([max(K_list[bb * GB:(bb + 1) * GB]) for bb in range(NGB)],
                  dtype=np.int64)
    boffs = np.concatenate([[0], np.cumsum(GB * Kb)]).astype(np.int64)
    totK = int(boffs[-1])          # padded slot total
    # padded slot offset of each tile
    soff = np.array([boffs[t // GB] + (t % GB) * Kb[t // GB]
                     for t in range(T_real)], dtype=np.int64)

    # per-edge slot assignment within (core, dst)
    eorder = np.lexsort((src, e_pos, e_owner))
    so, sp, ss = e_owner[eorder], e_pos[eorder], src[eorder]
    gkey = so * per + sp
    newg = np.ones(len(gkey), dtype=bool)
    newg[1:] = gkey[1:] != gkey[:-1]
    gstart = np.where(newg)[0]
    slot = np.arange(len(gkey)) - np.repeat(
        gstart, np.diff(np.concatenate([gstart, [len(gkey)]])))
    tt = sp // P
    lane = sp % P

    # idx: per-instruction wrapped int16 blocks (instructions span the whole
    # batch's GB*Kb k-units); mask: duplicated pairs for DVE 2x
    idx16 = np.zeros((C, 16, 8 * totK), np.int16)
    mask = np.zeros((C, P, 4 * totK), np.float32)
    ebb = tt // GB
    kk = (tt % GB) * Kb[ebb] + slot           # within-batch k-unit
    j8 = kk % KU
    chunk = kk // KU
    i_local = j8 * P + lane
    col = 8 * boffs[ebb] + chunk * (8 * KU) + i_local // 16
    rr = i_local % 16
    idx16[so, rr, col] = grp[ss]
    mask[so, lane, 4 * (soff[tt] + slot) + sub[ss]] = 1.0
    idx_arr = np.tile(idx16, (1, 8, 1))
    mask_arr = np.repeat(mask, 2, axis=2).astype(BF16NP)

    # per-instruction chunk table: (batch, col0, ku, k0)
    chunks = []
    for bb in range(NGB):
        nk = GB * int(Kb[bb])
        nfull, rem = divmod(nk, KU)
        for h in range(nfull):
            chunks.append((bb, 8 * boffs[bb] + h * 8 * KU, KU, h * KU))
        if rem:
            chunks.append((bb, 8 * boffs[bb] + nfull * 8 * KU, rem,
                           nfull * KU))

    # dinv per core in shard order -> [P, T_all]
    nodes_by_cp = np.full((C, per), -1, dtype=np.int64)
    nodes_by_cp[owner, pos] = np.arange(n_nodes)
    dinv_s = np.zeros((C, P, T_all), dtype=np.float32)
    for c in range(C):
        fulls = np.zeros(SH, np.float32)
        fulls[:per] = dinv[nodes_by_cp[c]]
        dinv_s[c] = fulls.reshape(T_all, P).T

    meta = dict(per=per, SH=SH, T_real=T_real, T_all=T_all, K_list=K_list,
                GB=GB, NGB=NGB, Kb=Kb, boffs=boffs, soff=soff,
                totK=totK, maxK=max(K_list), chunks=chunks,
                nodes_by_cp=nodes_by_cp)
    return idx_arr, mask_arr, dinv_s, meta


def _build_nc(n_nodes, meta, phases=("p1", "ag1", "l1", "ag2", "l2"), reps=1):
    phases = set(phases)
    SH, T_real, T_all = meta["SH"], meta["T_real"], meta["T_all"]
    totK, maxK = meta["totK"], meta["maxK"]
    TAB = C * SH

    nc = bacc.Bacc("TRN2", target_bir_lowering=False, debug=False,
                   num_devices=C, dynamic_dma_scratch_size=49152,
                   num_swdge_queues=4)
    xT = nc.dram_tensor("xT", [D, SH], F32, kind="ExternalInput").ap()
    w1 = nc.dram_tensor("w1", [D, H], F32, kind="ExternalInput").ap()
    w2b = nc.dram_tensor("w2b", [P, P], F32, kind="ExternalInput").ap()
    b1r = nc.dram_tensor("b1r", [P, TB * H], F32, kind="ExternalInput").ap()
    b2r = nc.dram_tensor("b2r", [P, TB * H], F32, kind="ExternalInput").ap()
    c2r = nc.dram_tensor("c2r", [P, TB * H], F32, kind="ExternalInput").ap()
    ident = nc.dram_tensor("ident", [P, P], F32, kind="ExternalInput").ap()
    dinvs = nc.dram_tensor("dinvs", [P, T_all], F32, kind="ExternalInput").ap()
    idx = nc.dram_tensor("idx", [P, 8 * totK], I16, kind="ExternalInput").ap()
    msk = nc.dram_tensor("msk", [P, 8 * totK], BF16, kind="ExternalInput").ap()
    out = nc.dram_tensor("out", [SH, H], F32, kind="ExternalOutput").ap()

    with tile.TileContext(nc) as tc:
        with (
            tc.tile_pool(name="cst", bufs=1) as cst,
            tc.tile_pool(name="sb", bufs=3) as sb,
            tc.tile_pool(name="gp", bufs=2) as gp,
            tc.tile_pool(name="fp", bufs=2) as fp,
            tc.tile_pool(name="xtp", bufs=2) as xtp,
            tc.tile_pool(name="ps", bufs=2, space="PSUM") as ps,
            tc.tile_pool(name="dram", bufs=1, space="DRAM") as dram,
        ):
            # constants
            w1t = cst.tile([P, 4 * H], F32)
            for f in range(4):
                nc.sync.dma_start(w1t[:, f * H:(f + 1) * H],
                                  w1[f * P:(f + 1) * P, :])
            w2bt = cst.tile([P, P], F32)
            nc.sync.dma_start(w2bt[:], w2b[:, :])
            b1t = cst.tile([P, TB * H], F32)
            nc.sync.dma_start(b1t[:], b1r[:, :])
            b2t = cst.tile([P, TB * H], F32)
            nc.sync.dma_start(b2t[:], b2r[:, :])
            c2t = cst.tile([P, TB * H], F32)
            nc.sync.dma_start(c2t[:], c2r[:, :])
            idt = cst.tile([P, P], F32)
            nc.sync.dma_start(idt[:], ident[:, :])
            dst_ = cst.tile([P, T_all], F32)
            nc.sync.dma_start(dst_[:], dinvs[:, :])
            ixt = cst.tile([P, 8 * totK], I16)
            nc.sync.dma_start(ixt[:], idx[:, :])
            mkt = cst.tile([P, 8 * totK], BF16)
            nc.sync.dma_start(mkt[:], msk[:, :])
            fz = cst.tile([1, H], BF16)
            nc.vector.memset(fz[:], 0.0)

            h1s = dram.tile([SH, H], BF16)
            h2s = dram.tile([SH, H], BF16)
            fsrc = dram.tile([1, H], BF16)
            nc.sync.dma_start(fsrc[:], fz[:])

            env = dict(locals())
            env["dram"] = dram
            for r in range(reps):
                tab1 = dram.tile([TAB, H], BF16, addr_space="Shared",
                                 tag=f"tab1_{r}")
                tab2 = dram.tile([TAB, H], BF16, addr_space="Shared",
                                 tag=f"tab2_{r}")
                env["tab1"], env["tab2"] = tab1, tab2
                env["rep"] = r
                _body(nc, tc, phases, meta, env)

    nc.compile()
    return nc


def _body(nc, tc, phases, meta, env):
    SH, T_real, T_all = meta["SH"], meta["T_real"], meta["T_all"]
    K_list, maxK = meta["K_list"], meta["maxK"]
    chunks = meta["chunks"]
    TAB = C * SH
    NST = SH // 512
    NB = T_all // TB           # batched epilogue groups

    xT = env["xT"]; out = env["out"]
    sb = env["sb"]; gp = env["gp"]; ps = env["ps"]
    fp = env["fp"]; xtp = env["xtp"]
    GB, NGB, Kb, boffs = meta["GB"], meta["NGB"], meta["Kb"], meta["boffs"]
    tab1 = env["tab1"]; tab2 = env["tab2"]
    h1s = env["h1s"]; h2s = env["h2s"]
    fsrc = env["fsrc"]
    w1t = env["w1t"]; w2bt = env["w2bt"]; b1t = env["b1t"]; b2t = env["b2t"]
    c2t = env["c2t"]; idt = env["idt"]; dst_ = env["dst_"]
    ixt = env["ixt"]; mkt = env["mkt"]

    qn = [0]

    def gather_batch(g, tabg, bb):
        g3 = g[:].rearrange("p (k e) -> p k e", e=P)
        for (ct, col0, ku, k0) in chunks:
            if ct != bb:
                continue
            ni = P * ku
            nc.gpsimd.dma_gather(
                g3[:, k0:k0 + ku, :], tabg,
                ixt[:, col0:col0 + 8 * ku], ni, ni, P,
                queue_num=qn[0] % 4)
            qn[0] += 1

    def masked_fold(g, redb, bb, j_base):
        """mask-mult (DVE 2x) + 2 packed folds for a whole gather batch,
        then one short strided reduce per tile."""
        nk = GB * int(Kb[bb])
        g4 = g[:, :nk * P].rearrange("p (j g e) -> p j g e", g=16, e=2)
        nc.vector.tensor_mul(
            g4, g4,
            mkt[:, 8 * boffs[bb]:8 * (boffs[bb] + nk)]
               .rearrange("p (j e) -> p j e", e=2).unsqueeze(2)
               .broadcast_to([P, 4 * nk, 16, 2]))
        gb_ = g[:, :nk * P].rearrange("p (j b f) -> p j b f", b=4, f=H)
        t1 = fp.tile([P, GB * maxK * 2 * H], BF16, tag="t1")
        t1v = t1[:, :nk * 2 * H].rearrange("p (j b f) -> p j b f", b=2, f=H)
        nc.vector.tensor_add(t1v, gb_[:, :, 0:2, :], gb_[:, :, 2:4, :])
        s1 = fp.tile([P, GB * maxK * H], BF16, tag="s1")
        s1v = s1[:, :nk * H].rearrange("p (j f) -> p j f", f=H)
        nc.vector.tensor_add(s1v, t1v[:, :, 0, :], t1v[:, :, 1, :])
        kb = int(Kb[bb])
        for j in range(GB):
            nc.vector.reduce_sum(
                out=redb[:, (j_base + j) * H:(j_base + j + 1) * H],
                in_=s1[:, j * kb * H:(j + 1) * kb * H]
                    .rearrange("p (j f) -> p f j", f=H),
                axis=mybir.AxisListType.X)

    def bcast_dinv(b):
        return (dst_[:, b * TB:(b + 1) * TB].unsqueeze(-1)
                .broadcast_to([P, TB, H]))

    def batch_rows(dr, b):
        """[p, j, f] DRAM-side AP of rows for tile-batch b."""
        return dr[b * TB * P:(b + 1) * TB * P, :].rearrange(
            "(j p) f -> p j f", j=TB)

    def v3(sbuf_ap):
        return sbuf_ap.rearrange("p (j f) -> p j f", f=H)

    def fence(tag):
        """Tiny AllGather; completion implies every core's preceding
        collective finished (in-order collective lane). The in-place int16
        add of its zero payload onto ixt makes later gathers RAW-depend on
        it."""
        fb = env["dram"].tile([C, H], BF16, tag=f"fb_{tag}")
        nc.gpsimd.collective_compute(
            "AllGather", mybir.AluOpType.bypass,
            replica_groups=[list(range(C))],
            ins=[fsrc.opt()], outs=[fb.opt()])
        fbt = sb.tile([P, 2], BF16, tag=f"fbt_{tag}")
        nc.sync.dma_start(fbt[:], fb[0:1, 0:2].to_broadcast((P, 2)))
        nc.vector.tensor_add(
            ixt[:], ixt[:],
            fbt[:, 0:1].bitcast(I16).broadcast_to([P, 8 * meta["totK"]]))

    # ---- P1 (sharded): h1 = (x @ W1) * dinv for my 12.8k rows ----
    for st in range(NST if "p1" in phases else 0):
        xt = xtp.tile([P, 4 * D], F32, tag="xt")
        for f in range(4):
            nc.sync.dma_start(
                xt[:, f * D:(f + 1) * D],
                xT[f * P:(f + 1) * P, st * 512:(st + 1) * 512])
        ht = sb.tile([P, 4 * H], BF16, tag="ht")
        for g4 in range(4):
            pp = ps.tile([P, H], F32, tag="p1ps")
            for f in range(4):
                nc.tensor.matmul(
                    pp[:],
                    lhsT=xt[:, f * D + g4 * P: f * D + (g4 + 1) * P],
                    rhs=w1t[:, f * H:(f + 1) * H],
                    start=(f == 0), stop=(f == 3))
            g = st * 4 + g4
            nc.scalar.activation(ht[:, g4 * H:(g4 + 1) * H], pp[:],
                                 mybir.ActivationFunctionType.Copy,
                                 scale=dst_[:, g:g + 1])
        nc.sync.dma_start(batch_rows(h1s, st), v3(ht[:]))

    # ---- AG1 + fence ----
    if "ag1" in phases:
        nc.gpsimd.collective_compute(
            "AllGather", mybir.AluOpType.bypass,
            replica_groups=[list(range(C))],
            ins=[h1s.opt()], outs=[tab1.opt()])
        fence(f"1_{env['rep']}")

    # ---- L1: aggregate + batched epilogue (BN folded into W2', c2) ----
    tab1g = tab1[:].rearrange("(g r) f -> g (r f)", r=4)
    NGPB = TB // GB
    for b in range(NB if "l1" in phases else 0):
        redb = sb.tile([P, TB * H], F32, tag="redb")
        nzero = 0
        for gi in range(NGPB):
            bb = b * NGPB + gi
            if bb >= NGB:
                nzero += GB
                continue
            g = gp.tile([P, GB * maxK * P], BF16, tag="g")
            gather_batch(g, tab1g, bb)
            masked_fold(g, redb, bb, gi * GB)
        if nzero:
            nc.vector.memset(redb[:, (TB - nzero) * H:], 0.0)
        sf = sb.tile([P, TB * H], BF16, tag="sf")
        nc.sync.dma_start(v3(sf[:]), batch_rows(h1s, b))
        nc.vector.tensor_add(redb[:], redb[:], sf[:])
        nc.vector.tensor_mul(
            redb[:].rearrange("p (j f) -> p j f", f=H),
            redb[:].rearrange("p (j f) -> p j f", f=H), bcast_dinv(b))
        nc.vector.tensor_add(redb[:], redb[:], b1t[:])
        nc.vector.tensor_scalar_max(redb[:], redb[:], 0.0)
        pt = ps.tile([P, P], F32, tag="pst")
        nc.tensor.transpose(pt[:], redb[:], idt[:])
        rt = sb.tile([P, P], F32, tag="rt")
        nc.scalar.activation(rt[:], pt[:],
                             mybir.ActivationFunctionType.Copy)
        p2 = ps.tile([P, P], F32, tag="ps2")
        nc.tensor.matmul(p2[:], lhsT=rt[:], rhs=w2bt[:],
                         start=True, stop=True)
        h2b = sb.tile([P, TB * H], F32, tag="h2b")
        nc.vector.tensor_add(h2b[:], p2[:], c2t[:])
        nc.vector.tensor_mul(
            h2b[:].rearrange("p (j f) -> p j f", f=H),
            h2b[:].rearrange("p (j f) -> p j f", f=H), bcast_dinv(b))
        h2c = sb.tile([P, TB * H], BF16, tag="h2c")
        nc.vector.tensor_copy(h2c[:], h2b[:])
        nc.sync.dma_start(batch_rows(h2s, b), v3(h2c[:]))

    # ---- AG2 + fence ----
    if "ag2" in phases:
        nc.gpsimd.collective_compute(
            "AllGather", mybir.AluOpType.bypass,
            replica_groups=[list(range(C))],
            ins=[h2s.opt()], outs=[tab2.opt()])
        fence(f"2_{env['rep']}")

    # ---- L2: aggregate + batched epilogue ----
    tab2g = tab2[:].rearrange("(g r) f -> g (r f)", r=4)
    for b in range(NB if "l2" in phases else 0):
        redb = sb.tile([P, TB * H], F32, tag="redb2")
        nzero = 0
        for gi in range(NGPB):
            bb = b * NGPB + gi
            if bb >= NGB:
                nzero += GB
                continue
            g = gp.tile([P, GB * maxK * P], BF16, tag="g")
            gather_batch(g, tab2g, bb)
            masked_fold(g, redb, bb, gi * GB)
        if nzero:
            nc.vector.memset(redb[:, (TB - nzero) * H:], 0.0)
        sf = sb.tile([P, TB * H], BF16, tag="sf2")
        nc.sync.dma_start(v3(sf[:]), batch_rows(h2s, b))
        nc.vector.tensor_add(redb[:], redb[:], sf[:])
        nc.vector.tensor_mul(
            redb[:].rearrange("p (j f) -> p j f", f=H),
            redb[:].rearrange("p (j f) -> p j f", f=H), bcast_dinv(b))
        nc.vector.tensor_add(redb[:], redb[:], b2t[:])
        nc.vector.tensor_scalar_max(redb[:], redb[:], 0.0)
        ot = sb.tile([P, TB * H], F32, tag="ot")
        nc.vector.tensor_copy(ot[:], redb[:])
        nc.sync.dma_start(batch_rows(out, b), v3(ot[:]))


def _impl(x, edge_index, W1, b1, W2, b2, gamma, beta, run_mean, run_var,
          n_nodes):
    x = np.asarray(x, np.float32)
    W1 = np.asarray(W1, np.float32)
    b1 = np.asarray(b1, np.float32)
    W2 = np.asarray(W2, np.float32)
    b2 = np.asarray(b2, np.float32)
    gamma = np.asarray(gamma, np.float32)
    beta = np.asarray(beta, np.float32)
    run_mean = np.asarray(run_mean, np.float32)
    run_var = np.asarray(run_var, np.float32)

    idx_arr, mask_arr, dinv_s, meta = _plan(n_nodes, np.asarray(edge_index))
    per, SH = meta["per"], meta["SH"]

    s = gamma / np.sqrt(run_var + BN_EPS)
    t = beta - run_mean * s
    W2p = (W2 * s[:, None]).astype(np.float32)
    c2 = (t @ W2).astype(np.float32)

    w2blk = np.zeros((P, P), np.float32)
    for j in range(TB):
        w2blk[j * H:(j + 1) * H, j * H:(j + 1) * H] = W2p
    b1rep = np.tile(b1[None, :], (P, TB)).astype(np.float32)
    b2rep = np.tile(b2[None, :], (P, TB)).astype(np.float32)
    c2rep = np.tile(c2[None, :], (P, TB)).astype(np.float32)
    identv = np.eye(P, dtype=np.float32)

    nodes_by_cp = meta["nodes_by_cp"]
    in_maps = []
    for c in range(C):
        xs = np.zeros((SH, D), np.float32)
        xs[:per] = x[nodes_by_cp[c]]
        in_maps.append({
            "xT": np.ascontiguousarray(xs.T), "w1": W1, "w2b": w2blk,
            "b1r": b1rep, "b2r": b2rep, "c2r": c2rep, "ident": identv,
            "dinvs": np.ascontiguousarray(dinv_s[c]),
            "idx": np.ascontiguousarray(idx_arr[c]),
            "msk": np.ascontiguousarray(mask_arr[c]),
        })

    nc = _build_nc(n_nodes, meta)
    global _LAST_NC, _LAST_IN_MAPS, _LAST_META
    _LAST_NC, _LAST_IN_MAPS, _LAST_META = nc, in_maps, meta
    res = run_bass_kernel_spmd(nc, in_maps, core_ids=list(range(C))).results

    outf = np.zeros((n_nodes, H), np.float32)
    for c in range(C):
        outf[nodes_by_cp[c]] = res[c]["out"][:per]
    return outf


def kernel(x, edge_index, W1, b1, W2, b2, gamma, beta, run_mean, run_var):
    return _impl(x, edge_index, W1, b1, W2, b2, gamma, beta, run_mean,
                 run_var, n_nodes=100000)
